# revision 16
# baseline (speedup 1.0000x reference)
"""Chunked causal attention (B=2, nh=16, Tq=1024, Tk=8192, dh=64) on 8 trn2 cores.

Strategy: shard (B*nh)=32 heads -> 4 heads/core, no cross-core comm.

Host-side prep (per head): K is cast to bf16 and laid out as a row-paired
K^T [128, 4096] (rows 0-63 = even k-tiles' [d, key], rows 64-127 = odd
k-tiles), Q^T [128, 1024] bf16 duplicated across both row halves, and
V | ones as [128 part(key%128), 64 tile, 65] fp32. This removes every
on-chip transpose/cast: each head needs just three large contiguous DMAs.

On-chip per (head, q-block of 512): for each group of 3 k-tiles,
S^T = K^T-pair-tiles @ Q^T on the PE (bf16, row-paired across array
halves), exp on ScalarE straight out of PSUM ([128, 1536] per call,
scale 1/8 fused), causal mask as a 0/1 multiply on diagonal groups only,
then PV with V|ones stationary so accumulator row 64 is the softmax
denominator. The [65, 512] accumulators go to DRAM raw; the divide and
final transpose happen on host.
"""

import base64
import io

import ml_dtypes
import numpy as np

import concourse.bacc as bacc
import concourse.bass as bass
import concourse.tile as tile
from concourse import mybir
from concourse.bass_utils import run_bass_kernel_spmd

F32 = mybir.dt.float32
F32R = mybir.dt.float32r
BF16 = mybir.dt.bfloat16

N_CORES = 8
B, NH, TQ, TK, D = 2, 16, 1024, 8192, 64
H = (B * NH) // N_CORES          # heads per core = 4
KT_TILES = TK // 128             # 64 k-tiles of 128
QB = TQ // 512                   # 2 q-blocks of 512

GSIZE = 3                        # k-tiles per exp batch ([128, 1536] PSUM)
GROUPS = [(GSIZE * g, GSIZE) for g in range(KT_TILES // GSIZE)]
if KT_TILES % GSIZE:
    GROUPS.append((GSIZE * (KT_TILES // GSIZE), KT_TILES % GSIZE))
GW = GSIZE * 512

# Schraudolph bit-trick exp for the groups offloaded to the DVE:
# int32 bits = round(s * EXP_A + EXP_B) reinterpreted as fp32 gives
# exp(s/sqrt(D)) within +-3.1% (c=0.03075 balanced max error).
EXP_A = float(2**23 * np.log2(np.e) / np.sqrt(D))
EXP_B = float(127.0 * 2**23 - round(0.03075 * 2**23))
DVE_NTH = 4                      # every 4th full group runs on the DVE


def _mask_info(q_chunk_start):
    """Per (group gi, q-block qb): status plus per-tile keep flags.
    status: 'full' | 'skip' | ('mask', idx). Masks padded to [128, GW] with
    ones. tile_keep[(gi, qb)] = list of per-tile 'any kept' bools."""
    info = {}
    tile_keep = {}
    masks = []
    for gi, (g0, ng) in enumerate(GROUPS):
        for qb in range(QB):
            qg = q_chunk_start + 512 * qb + np.arange(512)
            keeps = []
            for i in range(ng):
                kg = 128 * (g0 + i) + np.arange(128)
                keeps.append(qg[None, :] >= kg[:, None])     # [128, 512]
            cat = np.concatenate(keeps, axis=1)
            tile_keep[(gi, qb)] = [k.any() for k in keeps]
            if cat.all():
                info[(gi, qb)] = ("full", None)
            elif not cat.any():
                info[(gi, qb)] = ("skip", None)
            else:
                pad = np.ones((128, GW), dtype=np.float32)
                pad[:, :cat.shape[1]] = cat
                info[(gi, qb)] = ("mask", len(masks))
                masks.append(pad)
    mask_arr = (np.stack(masks) if masks
                else np.zeros((1, 128, GW), dtype=np.float32))
    return info, tile_keep, mask_arr


def _inline_f32r(nc, data, name):
    """inline_tensor, but declared float32r (same bits as float32) so plain
    HWDGE DMAs into float32r SBUF tiles need no gpsimd cast."""
    data = np.ascontiguousarray(data.astype(np.float32))
    mls = nc._tensor(name, list(data.shape), F32R, kind="Const", type="DRAM")
    buf = io.BytesIO()
    np.save(buf, data, allow_pickle=False)
    mls.file = f"{name}.npy"
    mls.ant_data = base64.standard_b64encode(buf.getvalue()).decode()
    return bass.DRamTensorHandle(name, list(data.shape), F32R)


def build_nc(q_chunk_start, loop_T=None):
    nc = bacc.Bacc("TRN2", target_bir_lowering=False, debug=False)

    # host-prepped inputs (see module docstring); f32r: FWL stays off (the
    # bf16 FWL weight path mis-loads at tile_position=(64,0))
    kt_d = nc.dram_tensor("ktp", [H, 128, KT_TILES // 2 * 128], F32R,
                          kind="ExternalInput")
    qt_d = nc.dram_tensor("qtp", [H, 128, TQ], F32R, kind="ExternalInput")
    # fp32 bits relabeled float32r for the PV matmul
    v_d = nc.dram_tensor("vp5", [H, 128, KT_TILES, D + 1], F32R,
                         kind="ExternalInput")
    o_d = nc.dram_tensor("o", [H, QB, 65, 512], F32, kind="ExternalOutput")

    info, tile_keep, mask_arr = _mask_info(q_chunk_start)
    n_masks = mask_arr.shape[0]
    masks_d = _inline_f32r(nc, mask_arr, "cmasks")

    with tile.TileContext(nc) as tc:
        with (
            tc.tile_pool(name="const", bufs=1) as const,
            tc.tile_pool(name="ktp", bufs=2) as ktp,
            tc.tile_pool(name="qtp", bufs=2) as qtp,
            tc.tile_pool(name="vp", bufs=2) as vp,
            tc.tile_pool(name="xp", bufs=4) as xp,
            tc.tile_pool(name="yp", bufs=2) as yp,
            tc.tile_pool(name="ostage", bufs=4) as ostage,
            tc.tile_pool(name="s_ps", bufs=2, space="PSUM") as s_ps,
            tc.tile_pool(name="o_ps", bufs=2, space="PSUM") as o_ps,
        ):
            mask_sb = const.tile([128, n_masks, GW], F32R)

            def load_masks():
                nc.sync.dma_start(
                    mask_sb[:], masks_d.ap().rearrange("m p f -> p m f"))

            # per-head persistent tiles, filled by interleaved prologue steps
            kt = {}    # h -> [128, 4096] bf16 row-paired K^T
            qt = {}    # h -> [128, TQ] bf16 duplicated Q^T
            vsb = {}   # h -> [128, KT_TILES, D+1] f32r V | ones

            def prologue_steps(h):
                steps = []
                kt[h] = ktp.tile([128, KT_TILES // 2 * 128], F32R,
                                 tag="kt", name=f"kt{h}")
                qt[h] = qtp.tile([128, TQ], F32R, tag="qt", name=f"qt{h}")
                vsb[h] = vp.tile([128, KT_TILES, D + 1], F32R,
                                 tag="v", name=f"v{h}")

                def q_load(h=h):
                    nc.sync.dma_start(qt[h][:], qt_d[h])
                steps.append(q_load)
                for c in range(4):
                    def k_chunk(c=c, h=h):
                        nc.sync.dma_start(
                            kt[h][:, c * 1024:(c + 1) * 1024],
                            kt_d[h][:, c * 1024:(c + 1) * 1024])
                    steps.append(k_chunk)
                for c in range(4):
                    def v_chunk(c=c, h=h):
                        nc.sync.dma_start(
                            vsb[h][:, c * 16:(c + 1) * 16, :],
                            v_d[h][:, c * 16:(c + 1) * 16, :])
                    steps.append(v_chunk)
                return steps

            def release(h):
                del kt[h], qt[h], vsb[h]

            def main_pairs(h, qb):
                active = [gi for gi in range(len(GROUPS))
                          if info[(gi, qb)][0] != "skip"]
                n_pv = sum(sum(tile_keep[(gi, qb)]) for gi in active)
                o_acc = o_ps.tile([65, 512], F32, tag="oacc")
                pv_state = {"i": 0}

                def emit_pv(gi, x_t):
                    g0, ng = GROUPS[gi]
                    keep = tile_keep[(gi, qb)]
                    for i in range(ng):
                        if not keep[i]:
                            continue
                        nc.tensor.matmul(
                            o_acc[:],
                            vsb[h][:, g0 + i, :],
                            x_t[:, 512 * i:512 * (i + 1)],
                            start=(pv_state["i"] == 0),
                            stop=(pv_state["i"] == n_pv - 1),
                        )
                        pv_state["i"] += 1

                prev = None
                n_full = 0
                for gi in active:
                    g0, ng = GROUPS[gi]
                    kind, mask_i = info[(gi, qb)]
                    keep = tile_keep[(gi, qb)]
                    w = 512 * ng
                    s_t = s_ps.tile([128, GW], F32, tag="s")
                    for i in range(ng):
                        if not keep[i]:
                            continue
                        t = g0 + i
                        rb = 64 * (t % 2)
                        nc.tensor.matmul(
                            s_t[:, 512 * i:512 * (i + 1)],
                            kt[h][rb:rb + 64, 128 * (t // 2):128 * (t // 2 + 1)],
                            qt[h][rb:rb + 64, 512 * qb:512 * (qb + 1)],
                            start=True, stop=True, tile_position=(rb, 0),
                        )
                    x_t = xp.tile([128, GW], F32R, tag="x")
                    on_dve = kind == "full" and n_full % DVE_NTH == DVE_NTH - 1
                    if kind == "full":
                        n_full += 1
                    if on_dve:
                        y_t = yp.tile([128, GW], mybir.dt.int32, tag="y")
                        nc.vector.tensor_scalar(
                            y_t[:, 0:w], s_t[:, 0:w], EXP_A, EXP_B,
                            mybir.AluOpType.mult, mybir.AluOpType.add)
                        nc.vector.tensor_copy(
                            x_t[:, 0:w], y_t[:, 0:w].bitcast(F32R))
                    else:
                        nc.scalar.activation(
                            x_t[:, 0:w], s_t[:, 0:w],
                            mybir.ActivationFunctionType.Exp,
                            scale=1.0 / np.sqrt(D),
                        )
                    if kind == "mask":
                        nc.vector.tensor_mul(
                            x_t[:, 0:w], x_t[:, 0:w], mask_sb[:, mask_i, 0:w])
                    if prev is not None:
                        emit_pv(*prev)
                    prev = (gi, x_t)
                    yield
                emit_pv(*prev)
                # epilogue: raw [65, 512] accumulator to DRAM;
                # divide-by-denominator + transpose happen on host.
                osb = ostage.tile([65, 512], F32, tag="osb")
                nc.vector.tensor_copy(osb[:], o_acc[:])
                nc.sync.dma_start(o_d[h, qb], osb[:])
                yield

            # ---- emission: minimal upfront prologue, then each head's
            # main loop with the successor prologue woven in
            import contextlib
            loop_ctx = (tc.For_i(0, loop_T, 1) if loop_T
                        else contextlib.nullcontext())
            loop_ctx.__enter__()
            first = prologue_steps(0)
            # [q, k0..k3, v0..v3]: q + first k chunk + first v chunk upfront
            upfront = [first[0], first[1], first[5]]
            woven0 = [first[2], first[6], first[3], first[7], first[4],
                      first[8], load_masks]
            for step in upfront:
                step()
            pend = {0: woven0}
            for h in range(H):
                steps = pend.pop(h, [])
                if h + 1 < H:
                    steps = steps + prologue_steps(h + 1)
                si = 0
                tick = 0
                stride = 2 if h == 0 else 4
                for qb in range(QB):
                    for _ in main_pairs(h, qb):
                        if tick % stride == 0 and si < len(steps):
                            steps[si]()
                            si += 1
                        tick += 1
                while si < len(steps):
                    steps[si]()
                    si += 1
                release(h)
            loop_ctx.__exit__(None, None, None)
    nc.compile()
    return nc


_CACHE = {}


def _get_nc(q_chunk_start):
    key = int(q_chunk_start)
    if key not in _CACHE:
        _CACHE[key] = build_nc(key)
    return _CACHE[key]


def kernel(q, k, v, q_chunk_start, _trace=False):
    q = np.ascontiguousarray(np.asarray(q, dtype=np.float32)).reshape(B * NH, TQ, D)
    k = np.ascontiguousarray(np.asarray(k, dtype=np.float32)).reshape(B * NH, TK, D)
    v = np.ascontiguousarray(np.asarray(v, dtype=np.float32)).reshape(B * NH, TK, D)
    qcs = int(np.asarray(q_chunk_start))

    # host-side layout prep (see module docstring)
    # K^T row-paired: [AH, 32 pair, 2 t2, 128 p, 64 d] -> [AH, (t2 d), (pair p)]
    AH = B * NH
    ktp = np.ascontiguousarray(
        k.reshape(AH, KT_TILES // 2, 2, 128, D)
        .transpose(0, 2, 4, 1, 3)
        .reshape(AH, 128, KT_TILES // 2 * 128))
    qtT = q.transpose(0, 2, 1)                                # [AH, 64, TQ]
    qtp = np.concatenate([qtT, qtT], axis=1)                  # [AH, 128, TQ]
    v5 = np.concatenate(
        [v, np.ones((AH, TK, 1), np.float32)], axis=2)        # [AH, TK, 65]
    vp5 = (v5.reshape(AH, KT_TILES, 128, D + 1)
           .transpose(0, 2, 1, 3))                            # [AH, 128, 64, 65]

    nc = _get_nc(qcs)
    in_maps = []
    for c in range(N_CORES):
        s = slice(c * H, (c + 1) * H)
        in_maps.append({
            "ktp": np.ascontiguousarray(ktp[s]),
            "qtp": np.ascontiguousarray(qtp[s]),
            "vp5": np.ascontiguousarray(vp5[s]),
        })
    res = run_bass_kernel_spmd(
        nc, in_maps, core_ids=list(range(N_CORES)), trace=_trace)
    raw = np.stack([res.results[c]["o"] for c in range(N_CORES)])
    # raw: [cores, H, QB, 65, 512]; row 64 is the softmax denominator
    num = raw[:, :, :, 0:D, :]
    den = raw[:, :, :, D:D + 1, :]
    out = (num / den).transpose(0, 1, 2, 4, 3)          # [c, H, QB, 512, D]
    out = out.reshape(B, NH, TQ, D)
    if _trace:
        kernel._last_exec_time_ns = res.exec_time_ns
        kernel._last_results = res
    return out


# revision 20
# speedup vs baseline: 1.0878x; 1.0878x over previous
"""Chunked causal attention (B=2, nh=16, Tq=1024, Tk=8192, dh=64) on 8 trn2 cores.

Strategy: shard (B*nh)=32 heads -> 4 heads/core, no cross-core comm.

Host-side prep (per head): K is cast to bf16 and laid out as a row-paired
K^T [128, 4096] (rows 0-63 = even k-tiles' [d, key], rows 64-127 = odd
k-tiles), Q^T [128, 1024] bf16 duplicated across both row halves, and
V | ones as [128 part(key%128), 64 tile, 65] fp32. This removes every
on-chip transpose/cast: each head needs just three large contiguous DMAs.

On-chip per (head, q-block of 512): for each group of 3 k-tiles,
S^T = K^T-pair-tiles @ Q^T on the PE (bf16, row-paired across array
halves), exp on ScalarE straight out of PSUM ([128, 1536] per call,
scale 1/8 fused), causal mask as a 0/1 multiply on diagonal groups only,
then PV with V|ones stationary so accumulator row 64 is the softmax
denominator. The [65, 512] accumulators go to DRAM raw; the divide and
final transpose happen on host.
"""

import base64
import io

import ml_dtypes
import numpy as np

import concourse.bacc as bacc
import concourse.bass as bass
import concourse.tile as tile
from concourse import mybir
from concourse.bass_utils import run_bass_kernel_spmd

F32 = mybir.dt.float32
F32R = mybir.dt.float32r
BF16 = mybir.dt.bfloat16

N_CORES = 8
B, NH, TQ, TK, D = 2, 16, 1024, 8192, 64
H = (B * NH) // N_CORES          # heads per core = 4
KT_TILES = TK // 128             # 64 k-tiles of 128
QB = TQ // 512                   # 2 q-blocks of 512

GSIZE = 2                        # k-tiles per exp batch ([128, 1024] PSUM)
GROUPS = [(GSIZE * g, GSIZE) for g in range(KT_TILES // GSIZE)]
if KT_TILES % GSIZE:
    GROUPS.append((GSIZE * (KT_TILES // GSIZE), KT_TILES % GSIZE))
GW = GSIZE * 512

# Schraudolph bit-trick exp for the groups offloaded to the DVE:
# int32 bits = round(s * EXP_A + EXP_B) reinterpreted as fp32 gives
# exp(s/sqrt(D)) within +-3.1% (c=0.03075 balanced max error).
EXP_A = float(2**23 * np.log2(np.e) / np.sqrt(D))
EXP_B = float(127.0 * 2**23 - round(0.0580 * 2**23))
DVE_NTH = 4                      # every 4th full group runs on the DVE


def _mask_info(q_chunk_start):
    """Per (group gi, q-block qb): status plus per-tile keep flags.
    status: 'full' | 'skip' | ('mask', idx). Masks padded to [128, GW] with
    ones. tile_keep[(gi, qb)] = list of per-tile 'any kept' bools."""
    info = {}
    tile_keep = {}
    masks = []
    for gi, (g0, ng) in enumerate(GROUPS):
        for qb in range(QB):
            qg = q_chunk_start + 512 * qb + np.arange(512)
            keeps = []
            for i in range(ng):
                kg = 128 * (g0 + i) + np.arange(128)
                keeps.append(qg[None, :] >= kg[:, None])     # [128, 512]
            cat = np.concatenate(keeps, axis=1)
            tile_keep[(gi, qb)] = [k.any() for k in keeps]
            if cat.all():
                info[(gi, qb)] = ("full", None)
            elif not cat.any():
                info[(gi, qb)] = ("skip", None)
            else:
                pad = np.ones((128, GW), dtype=np.float32)
                pad[:, :cat.shape[1]] = cat
                info[(gi, qb)] = ("mask", len(masks))
                masks.append(pad)
    mask_arr = (np.stack(masks) if masks
                else np.zeros((1, 128, GW), dtype=np.float32))
    return info, tile_keep, mask_arr


def _inline_f32r(nc, data, name):
    """inline_tensor, but declared float32r (same bits as float32) so plain
    HWDGE DMAs into float32r SBUF tiles need no gpsimd cast."""
    data = np.ascontiguousarray(data.astype(np.float32))
    mls = nc._tensor(name, list(data.shape), F32R, kind="Const", type="DRAM")
    buf = io.BytesIO()
    np.save(buf, data, allow_pickle=False)
    mls.file = f"{name}.npy"
    mls.ant_data = base64.standard_b64encode(buf.getvalue()).decode()
    return bass.DRamTensorHandle(name, list(data.shape), F32R)


def build_nc(q_chunk_start, loop_T=None):
    nc = bacc.Bacc("TRN2", target_bir_lowering=False, debug=False)

    # host-prepped inputs (see module docstring); f32r: FWL stays off (the
    # bf16 FWL weight path mis-loads at tile_position=(64,0))
    kt_d = nc.dram_tensor("ktp", [H, 128, KT_TILES // 2 * 128], F32R,
                          kind="ExternalInput")
    qt_d = nc.dram_tensor("qtp", [H, 128, TQ], F32R, kind="ExternalInput")
    # fp32 bits relabeled float32r for the PV matmul
    v_d = nc.dram_tensor("vp5", [H, 128, KT_TILES, D + 1], F32R,
                         kind="ExternalInput")
    o_d = nc.dram_tensor("o", [H, QB, 65, 512], F32, kind="ExternalOutput")

    info, tile_keep, mask_arr = _mask_info(q_chunk_start)
    n_masks = mask_arr.shape[0]
    masks_d = _inline_f32r(nc, mask_arr, "cmasks")

    with tile.TileContext(nc) as tc:
        with (
            tc.tile_pool(name="const", bufs=1) as const,
            tc.tile_pool(name="ktp", bufs=2) as ktp,
            tc.tile_pool(name="qtp", bufs=2) as qtp,
            tc.tile_pool(name="vp", bufs=2) as vp,
            tc.tile_pool(name="xp", bufs=6) as xp,
            tc.tile_pool(name="yp", bufs=2) as yp,
            tc.tile_pool(name="ostage", bufs=4) as ostage,
            tc.tile_pool(name="s_ps", bufs=3, space="PSUM") as s_ps,
            tc.tile_pool(name="o_ps", bufs=2, space="PSUM") as o_ps,
        ):
            mask_sb = const.tile([128, n_masks, GW], F32R)

            def load_masks():
                nc.sync.dma_start(
                    mask_sb[:], masks_d.ap().rearrange("m p f -> p m f"))

            # per-head persistent tiles, filled by interleaved prologue steps
            kt = {}    # h -> [128, 4096] bf16 row-paired K^T
            qt = {}    # h -> [128, TQ] bf16 duplicated Q^T
            vsb = {}   # h -> [128, KT_TILES, D+1] f32r V | ones

            def prologue_steps(h):
                steps = []
                kt[h] = ktp.tile([128, KT_TILES // 2 * 128], F32R,
                                 tag="kt", name=f"kt{h}")
                qt[h] = qtp.tile([128, TQ], F32R, tag="qt", name=f"qt{h}")
                vsb[h] = vp.tile([128, KT_TILES, D + 1], F32R,
                                 tag="v", name=f"v{h}")

                def q_load(h=h):
                    nc.sync.dma_start(qt[h][:], qt_d[h])
                steps.append(q_load)
                for c in range(4):
                    def k_chunk(c=c, h=h):
                        nc.sync.dma_start(
                            kt[h][:, c * 1024:(c + 1) * 1024],
                            kt_d[h][:, c * 1024:(c + 1) * 1024])
                    steps.append(k_chunk)
                for c in range(4):
                    def v_chunk(c=c, h=h):
                        nc.sync.dma_start(
                            vsb[h][:, c * 16:(c + 1) * 16, :],
                            v_d[h][:, c * 16:(c + 1) * 16, :])
                    steps.append(v_chunk)
                return steps

            def release(h):
                del kt[h], qt[h], vsb[h]

            def main_pairs(h, qb):
                active = [gi for gi in range(len(GROUPS))
                          if info[(gi, qb)][0] != "skip"]
                n_pv = sum(sum(tile_keep[(gi, qb)]) for gi in active)
                o_acc = o_ps.tile([65, 512], F32, tag="oacc")
                pv_state = {"i": 0}

                def emit_pv(gi, x_t):
                    g0, ng = GROUPS[gi]
                    keep = tile_keep[(gi, qb)]
                    for i in range(ng):
                        if not keep[i]:
                            continue
                        nc.tensor.matmul(
                            o_acc[:],
                            vsb[h][:, g0 + i, :],
                            x_t[:, 512 * i:512 * (i + 1)],
                            start=(pv_state["i"] == 0),
                            stop=(pv_state["i"] == n_pv - 1),
                        )
                        pv_state["i"] += 1

                prev = None
                n_full = 0
                for gi in active:
                    g0, ng = GROUPS[gi]
                    kind, mask_i = info[(gi, qb)]
                    keep = tile_keep[(gi, qb)]
                    w = 512 * ng
                    s_t = s_ps.tile([128, GW], F32, tag="s")
                    for i in range(ng):
                        if not keep[i]:
                            continue
                        t = g0 + i
                        rb = 64 * (t % 2)
                        nc.tensor.matmul(
                            s_t[:, 512 * i:512 * (i + 1)],
                            kt[h][rb:rb + 64, 128 * (t // 2):128 * (t // 2 + 1)],
                            qt[h][rb:rb + 64, 512 * qb:512 * (qb + 1)],
                            start=True, stop=True, tile_position=(rb, 0),
                        )
                    x_t = xp.tile([128, GW], F32R, tag="x")
                    on_dve = kind == "full" and n_full % DVE_NTH == DVE_NTH - 1
                    if kind == "full":
                        n_full += 1
                    if on_dve:
                        y_t = yp.tile([128, GW], mybir.dt.int32, tag="y")
                        nc.vector.tensor_scalar(
                            y_t[:, 0:w], s_t[:, 0:w], EXP_A, EXP_B,
                            mybir.AluOpType.mult, mybir.AluOpType.add)
                        nc.vector.tensor_copy(
                            x_t[:, 0:w], y_t[:, 0:w].bitcast(F32R))
                    else:
                        nc.scalar.activation(
                            x_t[:, 0:w], s_t[:, 0:w],
                            mybir.ActivationFunctionType.Exp,
                            scale=1.0 / np.sqrt(D),
                        )
                    if kind == "mask":
                        nc.vector.tensor_mul(
                            x_t[:, 0:w], x_t[:, 0:w], mask_sb[:, mask_i, 0:w])
                    if prev is not None:
                        emit_pv(*prev)
                    prev = (gi, x_t)
                    yield
                emit_pv(*prev)
                # epilogue: raw [65, 512] accumulator to DRAM;
                # divide-by-denominator + transpose happen on host.
                osb = ostage.tile([65, 512], F32, tag="osb")
                nc.vector.tensor_copy(osb[:], o_acc[:])
                nc.sync.dma_start(o_d[h, qb], osb[:])
                yield

            # ---- emission: minimal upfront prologue, then each head's
            # main loop with the successor prologue woven in
            import contextlib
            loop_ctx = (tc.For_i(0, loop_T, 1) if loop_T
                        else contextlib.nullcontext())
            loop_ctx.__enter__()
            first = prologue_steps(0)
            # [q, k0..k3, v0..v3]: q + first k chunk + first v chunk upfront
            upfront = [first[0], first[1], first[5]]
            woven0 = [first[2], first[6], first[3], first[7], first[4],
                      first[8], load_masks]
            for step in upfront:
                step()
            pend = {0: woven0}
            for h in range(H):
                steps = pend.pop(h, [])
                if h + 1 < H:
                    steps = steps + prologue_steps(h + 1)
                si = 0
                tick = 0
                stride = 2 if h == 0 else 4
                for qb in range(QB):
                    for _ in main_pairs(h, qb):
                        if tick % stride == 0 and si < len(steps):
                            steps[si]()
                            si += 1
                        tick += 1
                while si < len(steps):
                    steps[si]()
                    si += 1
                release(h)
            loop_ctx.__exit__(None, None, None)
    nc.compile()
    return nc


_CACHE = {}


def _get_nc(q_chunk_start):
    key = int(q_chunk_start)
    if key not in _CACHE:
        _CACHE[key] = build_nc(key)
    return _CACHE[key]


def kernel(q, k, v, q_chunk_start, _trace=False):
    q = np.ascontiguousarray(np.asarray(q, dtype=np.float32)).reshape(B * NH, TQ, D)
    k = np.ascontiguousarray(np.asarray(k, dtype=np.float32)).reshape(B * NH, TK, D)
    v = np.ascontiguousarray(np.asarray(v, dtype=np.float32)).reshape(B * NH, TK, D)
    qcs = int(np.asarray(q_chunk_start))

    # host-side layout prep (see module docstring)
    # K^T row-paired: [AH, 32 pair, 2 t2, 128 p, 64 d] -> [AH, (t2 d), (pair p)]
    AH = B * NH
    ktp = np.ascontiguousarray(
        k.reshape(AH, KT_TILES // 2, 2, 128, D)
        .transpose(0, 2, 4, 1, 3)
        .reshape(AH, 128, KT_TILES // 2 * 128))
    qtT = q.transpose(0, 2, 1)                                # [AH, 64, TQ]
    qtp = np.concatenate([qtT, qtT], axis=1)                  # [AH, 128, TQ]
    v5 = np.concatenate(
        [v, np.ones((AH, TK, 1), np.float32)], axis=2)        # [AH, TK, 65]
    vp5 = (v5.reshape(AH, KT_TILES, 128, D + 1)
           .transpose(0, 2, 1, 3))                            # [AH, 128, 64, 65]

    nc = _get_nc(qcs)
    in_maps = []
    for c in range(N_CORES):
        s = slice(c * H, (c + 1) * H)
        in_maps.append({
            "ktp": np.ascontiguousarray(ktp[s]),
            "qtp": np.ascontiguousarray(qtp[s]),
            "vp5": np.ascontiguousarray(vp5[s]),
        })
    res = run_bass_kernel_spmd(
        nc, in_maps, core_ids=list(range(N_CORES)), trace=_trace)
    raw = np.stack([res.results[c]["o"] for c in range(N_CORES)])
    # raw: [cores, H, QB, 65, 512]; row 64 is the softmax denominator
    num = raw[:, :, :, 0:D, :]
    den = raw[:, :, :, D:D + 1, :]
    out = (num / den).transpose(0, 1, 2, 4, 3)          # [c, H, QB, 512, D]
    out = out.reshape(B, NH, TQ, D)
    if _trace:
        kernel._last_exec_time_ns = res.exec_time_ns
        kernel._last_results = res
    return out
